# revision 3
# baseline (speedup 1.0000x reference)
"""GTG replicator-dynamics kernel for 8 Trainium2 NeuronCores.

Strategy (row-shard; A resident in SBUF as fp16):
  - A = thresholded cosine-sim Gram matrix [8192, 8192]. Each core owns the
    column block S_c = A[:, rows_c] (= row block transposed; A symmetric),
    stored fp16 in SBUF (16 MB) so the 30 replicator iterations never re-read
    A from HBM.
  - Construction is f32 on the PE (normalize-first), bounced through DRAM so
    the mean-threshold compare happens on f32 values exactly like the
    reference; only the post-threshold store rounds to fp16.
  - Each iteration: AX^T[c, local_rows] = X^T @ S_c via PE (X [128,10] fp16
    stationary, S_c fp16 streamed), PE-transpose back to row-major, row
    normalize + entropy on DVE/ACT, AllGather the new X rows (fp16, 160 KB).
"""

import os
import sys

sys.path.insert(0, "/opt/trn_rl_repo")

import numpy as np

N = 8192
D = 512
C = 10
NCORES = 8
ROWS = N // NCORES          # 1024 rows per core
KT = N // 128               # 64 k-tiles
RB = ROWS // 128            # 8 row-blocks per core
ITERS = int(os.environ.get("GTG_ITERS", "30"))
EPS = 1e-8
INV_LOGC = float(np.float32(1.0) / np.log(np.float32(C)))

_CACHE = {}


def _build():
    import concourse.bacc as bacc
    import concourse.mybir as mybir
    import concourse.tile as tile

    dt = mybir.dt
    AF = mybir.ActivationFunctionType
    OP = mybir.AluOpType
    RG = [list(range(NCORES))]

    nc = bacc.Bacc("TRN2", target_bir_lowering=False, debug=False,
                   num_devices=NCORES)

    embT = nc.dram_tensor("embT", [D, N], dt.float32, kind="ExternalInput")
    emb = nc.dram_tensor("emb", [N, D], dt.float32, kind="ExternalInput")
    embL = nc.dram_tensor("embL", [ROWS, D], dt.float32, kind="ExternalInput")
    embTL = nc.dram_tensor("embTL", [D, ROWS], dt.float32, kind="ExternalInput")
    x0l = nc.dram_tensor("x0l", [ROWS, C], dt.float32, kind="ExternalInput")
    x0f = nc.dram_tensor("x0f", [N, C], dt.float16, kind="ExternalInput")
    ident = nc.dram_tensor("ident", [128, C], dt.float32, kind="ExternalInput")
    ones = nc.dram_tensor("ones", [128, 1], dt.float32, kind="ExternalInput")
    xh = nc.dram_tensor("xh", [ROWS, ITERS, C], dt.float32, kind="ExternalOutput")
    eh = nc.dram_tensor("eh", [ROWS, ITERS], dt.float32, kind="ExternalOutput")

    with tile.TileContext(nc) as tc:
        with tc.tile_pool(name="dram", bufs=1, space="DRAM") as dram, \
             tc.tile_pool(name="dram2", bufs=2, space="DRAM") as dram2, \
             tc.tile_pool(name="pp", bufs=1) as pp, \
             tc.tile_pool(name="psS", bufs=1, space="PSUM") as psS:

            Gbuf = dram.tile([128, KT, 2, 512], dt.float32)
            sown_d = dram.tile([ROWS], dt.float32)
            ar_in = dram.tile([1, 1], dt.float32)
            ar_out = dram.tile([1, 1], dt.float32, addr_space="Shared")

            id_sb = pp.tile([128, C], dt.float32)
            ones_sb = pp.tile([128, 1], dt.float32)
            thr_b = pp.tile([128, 1], dt.float32)
            eps_b = pp.tile([128, 1], dt.float32)
            nc.gpsimd.memset(eps_b[:], EPS)
            nc.sync.dma_start(id_sb[:], ident.ap())
            nc.sync.dma_start(ones_sb[:], ones.ap())

            # ---------------- C1: G = En @ En_local^T (f32) -> DRAM --------
            with tc.tile_pool(name="c1", bufs=1) as c1p, \
                 tc.tile_pool(name="c1s", bufs=3) as c1s, \
                 tc.tile_pool(name="psG", bufs=2, space="PSUM") as psG:

                # norms of all rows (from row-major tiles, fused square+sum)
                nsq = c1p.tile([128, KT], dt.float32)
                embr = emb.ap().rearrange("(t p) d -> p t d", p=128)
                for t in range(KT):
                    est = c1s.tile([128, D], dt.float32, tag="est")
                    nc.sync.dma_start(est[:], embr[:, t, :])
                    scr = c1s.tile([128, D], dt.float32, tag="scr")
                    nc.scalar.activation(scr[:], est[:], AF.Square,
                                         accum_out=nsq[:, t:t + 1])
                nsqo = c1p.tile([128, RB], dt.float32)
                emblr = embL.ap().rearrange("(t p) d -> p t d", p=128)
                for t in range(RB):
                    est = c1s.tile([128, D], dt.float32, tag="est")
                    nc.sync.dma_start(est[:], emblr[:, t, :])
                    scr = c1s.tile([128, D], dt.float32, tag="scr")
                    nc.scalar.activation(scr[:], est[:], AF.Square,
                                         accum_out=nsqo[:, t:t + 1])

                # s = 1/(sqrt(nsq) + eps), then broadcast across partitions
                nrm = c1p.tile([128, KT], dt.float32)
                nc.scalar.activation(nrm[:], nsq[:], AF.Sqrt)
                nc.vector.tensor_scalar_add(nrm[:], nrm[:], EPS)
                srow = c1p.tile([128, KT], dt.float32)
                nc.vector.reciprocal(srow[:], nrm[:])
                nrmo = c1p.tile([128, RB], dt.float32)
                nc.scalar.activation(nrmo[:], nsqo[:], AF.Sqrt)
                nc.vector.tensor_scalar_add(nrmo[:], nrmo[:], EPS)
                srowo = c1p.tile([128, RB], dt.float32)
                nc.vector.reciprocal(srowo[:], nrmo[:])

                nc.sync.dma_start(sown_d[:].rearrange("(t p) -> p t", p=128),
                                  srowo[:])
                s1o = c1p.tile([1, ROWS], dt.float32)
                nc.sync.dma_start(s1o[:1, :], sown_d[:])
                s_own_b = c1p.tile([128, ROWS], dt.float32)
                nc.gpsimd.partition_broadcast(s_own_b[:], s1o[:1, :])

                # raw transposed embeddings; normalization applied to G tiles
                # (row scale fused into the Relu evacuation, col scale one TT)
                ent4 = c1p.tile([128, 4, N], dt.float32)
                nc.sync.dma_start(ent4[:],
                                  embT.ap().rearrange("(t p) n -> p t n", p=128))
                entl = c1p.tile([128, 4, ROWS], dt.float32)
                nc.sync.dma_start(entl[:],
                                  embTL.ap().rearrange("(t p) n -> p t n", p=128))

                # G tiles: [128 global rows, 512 local cols]; relu-clamp;
                # per-tile free-dim sums for the global mean
                reds = c1p.tile([128, 128], dt.float32)
                idx = 0
                for m in range(KT):
                    for n in range(2):
                        pg = psG.tile([128, 512], dt.float32, tag="pg")
                        for k in range(4):
                            nc.tensor.matmul(
                                pg[:],
                                ent4[:, k, 128 * m:128 * m + 128],
                                entl[:, k, 512 * n:512 * n + 512],
                                start=(k == 0), stop=(k == 3))
                        stg = c1s.tile([128, 512], dt.float32, tag="stg")
                        nc.scalar.activation(stg[:], pg[:], AF.Relu,
                                             scale=srow[:, m:m + 1])
                        nc.vector.tensor_tensor(
                            stg[:], stg[:],
                            s_own_b[:, 512 * n:512 * n + 512], OP.mult)
                        nc.vector.tensor_reduce(reds[:, idx:idx + 1], stg[:],
                                                axis=mybir.AxisListType.X,
                                                op=OP.add)
                        nc.sync.dma_start(Gbuf[:, m, n, :], stg[:])
                        idx += 1

                acc = c1p.tile([128, 1], dt.float32)
                nc.vector.tensor_reduce(acc[:], reds[:],
                                        axis=mybir.AxisListType.X, op=OP.add)
                ssum_ps = psS.tile([1, 1], dt.float32)
                nc.tensor.matmul(ssum_ps[:], acc[:], ones_sb[:],
                                 start=True, stop=True)
                ssum_sb = c1p.tile([1, 1], dt.float32)
                nc.vector.tensor_copy(ssum_sb[:], ssum_ps[:])
                nc.sync.dma_start(ar_in[:], ssum_sb[:])
                nc.gpsimd.collective_compute(
                    "AllReduce", OP.add, replica_groups=RG,
                    ins=[ar_in[:].opt()], outs=[ar_out[:].opt()])
                tsum = c1p.tile([1, 1], dt.float32)
                nc.sync.dma_start(tsum[:], ar_out[:])
                thr_sb = c1p.tile([1, 1], dt.float32)
                nc.vector.tensor_scalar(thr_sb[:], tsum[:],
                                        1.0 / float(N) / float(N), None,
                                        op0=OP.mult)
                nc.gpsimd.partition_broadcast(thr_b[:], thr_sb[:1, :])

            # ---------------- C2 + iterations ------------------------------
            with tc.tile_pool(name="mp", bufs=1) as mp, \
                 tc.tile_pool(name="ms", bufs=2) as ms, \
                 tc.tile_pool(name="psA", bufs=1, space="PSUM") as psA, \
                 tc.tile_pool(name="psB", bufs=2, space="PSUM") as psB:

                A_sb = mp.tile([128, KT, 1024], dt.float16)

                # threshold on f32 values, store fp16
                for j in range(KT // 2):
                    g2 = ms.tile([128, 2, 1024], dt.float32, tag="g2")
                    nc.sync.dma_start(
                        g2[:],
                        Gbuf[:, 2 * j:2 * j + 2, :, :].rearrange(
                            "p a b n -> p a (b n)"))
                    msk = ms.tile([128, 2, 1024], dt.float32, tag="msk")
                    nc.vector.tensor_scalar(msk[:], g2[:], thr_b[:], None,
                                            op0=OP.is_ge)
                    nc.vector.tensor_tensor(A_sb[:, 2 * j:2 * j + 2, :],
                                            g2[:], msk[:], OP.mult)

                X_kt = mp.tile([128, KT, C], dt.float16)
                nc.sync.dma_start(X_kt[:],
                                  x0f.ap().rearrange("(k p) c -> p k c", p=128))
                xl = ms.tile([128, RB, C], dt.float32, tag="xl")
                nc.sync.dma_start(xl[:],
                                  x0l.ap().rearrange("(b p) c -> p b c", p=128))

                for t in range(ITERS):
                    axt_ps = psA.tile([C, 1024], dt.float32, tag="axt")
                    for k in range(KT):
                        for n in range(2):
                            nc.tensor.matmul(
                                axt_ps[:, 512 * n:512 * n + 512],
                                X_kt[:, k, :],
                                A_sb[:, k, 512 * n:512 * n + 512],
                                start=(k == 0), stop=(k == KT - 1))
                    axt_sb = ms.tile([C, 1024], dt.float32, tag="axt_sb")
                    nc.vector.tensor_copy(axt_sb[:], axt_ps[:])

                    ax_ps = psB.tile([128, RB, C], dt.float32, tag="ax")
                    for rb in range(RB):
                        nc.tensor.matmul(
                            ax_ps[:, rb, :],
                            axt_sb[:, 128 * rb:128 * rb + 128],
                            id_sb[:C, :],
                            is_transpose=True,
                            start=(rb == 0), stop=(rb == RB - 1))

                    U = ms.tile([128, RB, C], dt.float32, tag="U")
                    nc.vector.tensor_tensor(U[:], xl[:], ax_ps[:], OP.mult)
                    s = ms.tile([128, RB], dt.float32, tag="s")
                    nc.vector.tensor_reduce(s[:], U[:],
                                            axis=mybir.AxisListType.X,
                                            op=OP.add)
                    nc.vector.tensor_scalar_add(s[:], s[:], EPS)
                    r = ms.tile([128, RB], dt.float32, tag="r")
                    nc.vector.reciprocal(r[:], s[:])
                    xn = ms.tile([128, RB, C], dt.float32, tag="xn")
                    for rb in range(RB):
                        nc.vector.tensor_scalar(xn[:, rb, :], U[:, rb, :],
                                                r[:, rb:rb + 1], None,
                                                op0=OP.mult)
                    xnh = ms.tile([128, RB, C], dt.float16, tag="xnh")
                    nc.vector.tensor_copy(xnh[:], xn[:])

                    cc_in = dram2.tile([ROWS, C], dt.float16, tag="cci")
                    nc.sync.dma_start(
                        cc_in[:].rearrange("(b p) c -> p b c", p=128), xnh[:])
                    cc_out = dram2.tile([N, C], dt.float16, tag="cco",
                                        addr_space="Shared")
                    nc.gpsimd.collective_compute(
                        "AllGather", OP.bypass, replica_groups=RG,
                        ins=[cc_in[:].opt()], outs=[cc_out[:].opt()])
                    if t < ITERS - 1:
                        nc.sync.dma_start(
                            X_kt[:],
                            cc_out[:].rearrange("(k p) c -> p k c", p=128))

                    L = ms.tile([128, RB, C], dt.float32, tag="L")
                    nc.scalar.activation(L[:], xn[:], AF.Ln, bias=eps_b[:])
                    P = ms.tile([128, RB, C], dt.float32, tag="P")
                    nc.vector.tensor_tensor(P[:], xn[:], L[:], OP.mult)
                    er = ms.tile([128, RB], dt.float32, tag="er")
                    nc.vector.tensor_reduce(er[:], P[:],
                                            axis=mybir.AxisListType.X,
                                            op=OP.add)
                    ent = ms.tile([128, RB], dt.float32, tag="ent")
                    nc.vector.tensor_scalar(ent[:], er[:], -INV_LOGC, None,
                                            op0=OP.mult)

                    nc.sync.dma_start(
                        xh.ap()[:, t, :].rearrange("(b p) c -> p b c", p=128),
                        xn[:])
                    nc.sync.dma_start(
                        eh.ap()[:, t].rearrange("(b p) -> p b", p=128),
                        ent[:])
                    xl = xn

    nc.compile()
    return nc


def kernel(embedding, lab_labels, n_lab_obs):
    from concourse.bass_utils import run_bass_kernel_spmd

    embedding = np.asarray(embedding, dtype=np.float32)
    labels = np.asarray(lab_labels).astype(np.int64)
    nlab = int(n_lab_obs)

    if "nc" not in _CACHE:
        _CACHE["nc"] = _build()
    nc = _CACHE["nc"]

    embT = np.ascontiguousarray(embedding.T)
    X0 = np.full((N, C), 1.0 / C, dtype=np.float32)
    X0[:nlab] = 0.0
    X0[np.arange(nlab), labels[:nlab]] = 1.0
    x0f = X0.astype(np.float16)
    ident = np.zeros((128, C), dtype=np.float32)
    ident[:C, :C] = np.eye(C, dtype=np.float32)
    ones = np.ones((128, 1), dtype=np.float32)

    in_maps = []
    for c in range(NCORES):
        r0 = c * ROWS
        in_maps.append({
            "embT": embT,
            "emb": embedding,
            "embL": np.ascontiguousarray(embedding[r0:r0 + ROWS]),
            "embTL": np.ascontiguousarray(embT[:, r0:r0 + ROWS]),
            "x0l": np.ascontiguousarray(X0[r0:r0 + ROWS]),
            "x0f": x0f,
            "ident": ident,
            "ones": ones,
        })

    trace = bool(int(os.environ.get("GTG_TRACE", "0")))
    res = run_bass_kernel_spmd(nc, in_maps, core_ids=list(range(NCORES)),
                               trace=trace)
    _CACHE["last_result"] = res

    X_hist = np.concatenate([res.results[c]["xh"] for c in range(NCORES)],
                            axis=0)
    ent_hist = np.concatenate([res.results[c]["eh"] for c in range(NCORES)],
                              axis=0)
    return X_hist, ent_hist
